# revision 29
# baseline (speedup 1.0000x reference)
"""BinaryMemoryRNNCell Trainium kernel v8.

Batch-sharded over 8 cores (16 rows/core), no collectives. bf16-only
weights/activations (8MB weight stream, 64 N=512 matmuls). Weights
stream as 1MB chunks in exact consumption order (x,h for both n-halves,
then hr,hl), alternating across both HWDGE queues. The index/gather
path (fp32 logits -> bits -> one 32-row SWDGE gather) runs in front of
the stream shadow; psA[0] finishes at the 6MB point so its stats
overlap the tail matmuls. Stats/normalize run directly on the [16,*]
layout: tensor_tensor_reduce fuses evacuation with sum/sumsq, then a
12-op bit-magic Newton rsqrt chain, normalize, sigmoid, split y DMAs.
"""
import numpy as np
import ml_dtypes
import concourse.bass as bass
from concourse import mybir
from concourse.bass import IndirectOffsetOnAxis
from concourse.bass_utils import run_bass_kernel_spmd


# ---------------------------------------------------------------------------
# Tile workarounds for this container's walrus build (max ONE sync wait per
# instruction): split the exit-drain's waits across single-wait NOPs, and a
# post-lowering pass that does the same for every multi-wait instruction.
# ---------------------------------------------------------------------------
import concourse.tile as _tile
from concourse.vector_clock import ScopedClock, VectorClock
from concourse.tile_sem_assignment import N_PROCS


class TileContextSplitDrain(_tile.TileContext):
    def _drain_and_barrier(self, tick_clock, wait_clock):
        gc = tick_clock.global_clock
        vals = [gc[p] for p in range(N_PROCS)]
        for base in range(N_PROCS):
            chunk_vals = [vals[p] if p == base else 0 for p in range(N_PROCS)]
            if not any(chunk_vals):
                continue
            nop_inst = self.nc.sync.nop(nofuse=True)
            wait_clock.add_sem_waits(
                nop_inst.ins, ScopedClock({None: VectorClock(chunk_vals)})
            )
        # The NOPs above (same engine, program order) already waited on the
        # full global clock; the drain needs no waits of its own.
        self.nc.sync.drain()
        self.nc.all_engine_barrier()
        assert self.sems is not None
        popped = self.nc._tile_sem_poison_stack.pop()
        assert popped is self._sem_poison
        self.nc.clear_and_free_semaphores(list(self.sems.allocated().values()))
        self.nc.all_engine_barrier()


def split_multi_waits(nc, max_waits=1):
    counter = 0
    for func in nc.m.functions:
        for bb in func.blocks:
            out = []
            changed = False
            for inst in bb.instructions:
                si = inst.sync_info
                if si is not None and len(si.on_wait) > max_waits:
                    waits = list(si.on_wait)
                    for w in waits[:-max_waits]:
                        counter += 1
                        out.append(mybir.InstNoOp(
                            name=f"waitsplit_{counter}",
                            engine=inst.engine,
                            bass_nofuse=True,
                            sync_info=mybir.SyncInfo(on_wait=[w], on_update=[]),
                        ))
                    inst.sync_info = mybir.SyncInfo(
                        on_wait=waits[-max_waits:], on_update=list(si.on_update))
                    changed = True
                out.append(inst)
            if changed:
                bb.instructions = out
    return counter

F32 = mybir.dt.float32
BF16 = mybir.dt.bfloat16
I32 = mybir.dt.int32
AO = mybir.AluOpType

NC = 8
B = 128
BL = 16
H = 1024
T = 1024
NB = 10
LN_EPS = 1e-5

_CACHED = {}


def _chunked_T(a):
    K, M = a.shape
    out = a.reshape(K // 128, 128, M).transpose(1, 0, 2).reshape(128, (K // 128) * M)
    return np.ascontiguousarray(out)


def _split_bf16(a):
    hi = a.astype(ml_dtypes.bfloat16)
    lo = (a - hi.astype(np.float32)).astype(ml_dtypes.bfloat16)
    return hi, lo


def _n0_first(w):
    """[128, 8k*1024] k-major -> [128, 2nt*4096] n-half major."""
    v = w.reshape(128, 8, 2, 512)
    return np.ascontiguousarray(v.transpose(0, 2, 1, 3).reshape(128, 8192))


def build(split=True, ln_trivial=False):
    nc = bass.Bass()
    p = {}
    p["mem"] = nc.declare_dram_parameter("mem", [T * BL, H], F32, isOutput=False)
    for m in range(4):
        p[f"wh{m}"] = nc.declare_dram_parameter(f"wh{m}", [128, 8192], BF16, isOutput=False)
    # critical consts: hT | mwt | id16/powmat/iota/mbrow (f32)
    p["cfc"] = nc.declare_dram_parameter("cfc", [128, 344], F32, isOutput=False)
    # act bf16 (xT hi, hT hi) + onesb
    p["cbc"] = nc.declare_dram_parameter("cbc", [128, 256], BF16, isOutput=False)
    p["bpair"] = nc.declare_dram_parameter("bpair", [8, H + 16], BF16, isOutput=False)
    if not ln_trivial:
        p["lngb"] = nc.declare_dram_parameter("lngb", [BL, 2 * H], F32, isOutput=False)
    y = nc.declare_dram_parameter("y", [BL, H], F32, isOutput=True)

    with TileContextSplitDrain(nc) as tc:
        with (
            tc.tile_pool(name="const", bufs=1) as cpool,
            tc.tile_pool(name="work", bufs=1) as wk,
            tc.tile_pool(name="wts", bufs=1) as wpool,
            tc.tile_pool(name="psum_small", bufs=3, space="PSUM") as psmall,
            tc.tile_pool(name="psum_main", bufs=1, space="PSUM") as pmain,
        ):
            # ---- critical const DMAs first: cfc+cbc on sync (clean queue),
            # bpr alone on scalar ----
            cfc = cpool.tile([128, 344], F32, name="cfc")
            cbc = cpool.tile([128, 256], BF16, name="cbc")
            bpr = cpool.tile([8, H + 16], BF16, name="bpr")
            nc.sync.dma_start(cfc[:], p["cfc"][:])
            nc.sync.dma_start(cbc[:], p["cbc"][:])
            nc.scalar.dma_start(bpr[:], p["bpair"][:])
            sb = {
                "hT": cfc[:, 0:128],
                "mwt": cfc[:, 128:288],
                "id16": cfc[0:16, 288:304],
                "powmat": cfc[0:20, 304:306],
                "iota": cfc[0:16, 306:307],
                "mbrow": cfc[0:1, 307:327],
                "xp": cbc[:, 0:128],
                "hp": cbc[:, 128:256],
                "onesb": None,  # set below from bpr
                "ones1": cfc[0:1, 327:343],
            }

            # ---- weight stream: 1MB chunks in consumption order; the last
            # two units split to 512KB so the final matmuls trail the stream
            # end by less ----
            wtiles = [wpool.tile([128, 8192], BF16, name=f"wh{m}") for m in range(4)]
            units = [(0, 0), (1, 0), (0, 1), (1, 1), (2, 0), (3, 0), (2, 1), (3, 1)]
            chunks = []
            for ui, (m, nt) in enumerate(units):
                base = nt * 4096
                if ui < 6:
                    chunks.append((m, base, base + 4096))
                elif ui == 6:
                    chunks.append((m, base, base + 2048))
                    chunks.append((m, base + 2048, base + 4096))
                else:
                    for q in range(4):
                        chunks.append((m, base + q * 1024, base + (q + 1) * 1024))
            # last chunk: peel off the final 8 columns as a tiny canary DMA on
            # the same engine (FIFO) — its completion receipt fires sooner
            # than the big chunk's, unblocking the final matmul earlier
            m_l, lo_l, hi_l = chunks[-1]
            chunks[-1] = (m_l, lo_l, hi_l - 8)
            chunks.append((m_l, hi_l - 8, hi_l))
            for ci, (m, lo, hi) in enumerate(chunks):
                if ci == len(chunks) - 1:
                    eng = nc.sync if (ci - 1) % 2 == 0 else nc.scalar
                else:
                    eng = nc.sync if ci % 2 == 0 else nc.scalar
                eng.dma_start(wtiles[m][:, lo:hi], p[f"wh{m}"][:, lo:hi])
            if not ln_trivial:
                lngb = cpool.tile([BL, 2 * H], F32, name="lngb")
                nc.scalar.dma_start(lngb[:], p["lngb"][:])

            # ---- logits -> bits -> flat gather indices (fp32) ----
            ps_lg = psmall.tile([BL, 2 * NB], F32, tag="small")
            nc.tensor.matmul(ps_lg[:], lhsT=sb["ones1"],
                             rhs=sb["mbrow"], start=True, stop=False)
            for k in range(8):
                nc.tensor.matmul(
                    ps_lg[:],
                    lhsT=sb["hT"][:, k * BL:(k + 1) * BL],
                    rhs=sb["mwt"][:, k * 2 * NB:(k + 1) * 2 * NB],
                    start=False, stop=(k == 7),
                )
            bits = wk.tile([BL, 2 * NB], F32)
            nc.vector.tensor_scalar(bits[:], ps_lg[:], 0.0, None, AO.is_gt)
            ps_bt = psmall.tile([2 * NB, BL], F32, tag="small")
            nc.tensor.transpose(ps_bt[:], bits[:], sb["id16"])
            bitsT = wk.tile([2 * NB, BL], F32)
            nc.vector.tensor_copy(bitsT[:], ps_bt[:])
            ps_idx = psmall.tile([BL, 2], F32, tag="small")
            nc.tensor.matmul(ps_idx[:], lhsT=bitsT[:], rhs=sb["powmat"],
                             start=True, stop=True)
            flatf = wk.tile([BL, 2], F32)
            nc.vector.tensor_scalar(flatf[:], ps_idx[:], float(BL), sb["iota"],
                                    AO.mult, AO.add)
            flati = wk.tile([BL, 2], I32)
            nc.vector.tensor_copy(flati[:], flatf[:])

            # ---- gathers (SWDGE) ----
            hrhl = wk.tile([BL, 2, H], F32)
            nc.gpsimd.indirect_dma_start(
                out=hrhl[:, 0, :], out_offset=None, in_=p["mem"][:],
                in_offset=IndirectOffsetOnAxis(ap=flati[:, 0:1], axis=0))
            nc.gpsimd.indirect_dma_start(
                out=hrhl[:, 1, :], out_offset=None, in_=p["mem"][:],
                in_offset=IndirectOffsetOnAxis(ap=flati[:, 1:2], axis=0))

            # ---- ACT table pre-warm (sigmoid) ----
            warm = wk.tile([1, 1], F32)
            nc.vector.memset(warm[:], 0.25)
            warm2 = wk.tile([1, 1], F32)
            nc.scalar.activation(warm2[:], warm[:], mybir.ActivationFunctionType.Sigmoid)

            # ---- main matmuls ----
            psA = [pmain.tile([BL, 512], F32, tag=f"A{nt}", name=f"psA{nt}")
                   for nt in range(2)]
            for nt in range(2):
                nc.tensor.matmul(psA[nt][:], lhsT=bpr[:, H:H + 16],
                                 rhs=bpr[:, nt * 512:(nt + 1) * 512],
                                 start=True, stop=False)
            memp = wk.tile([128, 2, 128], BF16)
            lhs_all = [sb["xp"], sb["hp"], memp[:, 0, :], memp[:, 1, :]]

            def mm_pair(ms, nt, stop_m=None):
                for m in ms:
                    th = wtiles[m]
                    lp = lhs_all[m]
                    for k in range(8):
                        stop = (m == stop_m) and (k == 7)
                        rs = slice(nt * 4096 + k * 512, nt * 4096 + k * 512 + 512)
                        nc.tensor.matmul(psA[nt][:],
                                         lhsT=lp[:, 16 * k:16 * (k + 1)],
                                         rhs=th[:, rs], start=False, stop=stop)

            mm_pair((0, 1), 0)
            mm_pair((0, 1), 1)

            # ---- transpose gathered rows -> bf16 chunks ----
            for j in range(2):
                for k in range(8):
                    ps_t = psmall.tile([128, BL], F32, tag="small", name=f"ps_t{k}")
                    nc.tensor.transpose(ps_t[:], hrhl[:, j, k * 128:(k + 1) * 128],
                                        sb["id16"])
                    nc.vector.tensor_copy(memp[:, j, 16 * k:16 * (k + 1)], ps_t[:])

            pre16 = wk.tile([BL, H], F32)
            sq16 = wk.tile([BL, H], F32)
            sacc = [wk.tile([BL, 1], F32, name=f"sacc{i}") for i in range(4)]

            def stats_half(nt):
                # vector: evacuate + row-sum; ACT: square + row-sumsq (reads
                # PSUM directly, runs in parallel with the vector pair)
                cs = slice(nt * 512, (nt + 1) * 512)
                nc.vector.tensor_copy(pre16[:, cs], psA[nt][:])
                nc.scalar.activation(sq16[:, cs], psA[nt][:],
                                     mybir.ActivationFunctionType.Square,
                                     accum_out=sacc[2 + nt][:])
                nc.vector.reduce_sum(sacc[nt][:], pre16[:, cs],
                                     axis=mybir.AxisListType.X)

            mm_pair((2, 3), 0, stop_m=3)
            stats_half(0)
            mm_pair((2, 3), 1, stop_m=3)
            stats_half(1)
            # reload the sigmoid ACT table while the rsqrt chain runs on DVE
            nc.scalar.activation(warm2[:], warm[:],
                                 mybir.ActivationFunctionType.Sigmoid)

            # ---- LayerNorm scalars: mu, var, bit-magic Newton rsqrt ----
            mu = wk.tile([BL, 1], F32)
            msq = wk.tile([BL, 1], F32)
            var = wk.tile([BL, 1], F32)
            yv = wk.tile([BL, 1], F32)
            t1 = wk.tile([BL, 1], F32)
            inv = wk.tile([BL, 1], F32)
            nc.vector.tensor_scalar(mu[:], sacc[0][:], sacc[1][:, 0:1], 1.0 / H,
                                    AO.add, AO.mult)
            nc.vector.tensor_scalar(msq[:], sacc[2][:], sacc[3][:, 0:1], 1.0 / H,
                                    AO.add, AO.mult)
            nc.vector.tensor_scalar(var[:], mu[:], mu[:, 0:1], None, AO.mult)
            nc.vector.tensor_scalar(var[:], msq[:], var[:, 0:1], LN_EPS,
                                    AO.subtract, AO.add)
            nc.vector.tensor_scalar(yv[:].bitcast(I32), var[:].bitcast(I32), 1, None,
                                    AO.logical_shift_right)
            nc.vector.tensor_scalar(yv[:].bitcast(I32), yv[:].bitcast(I32), -1,
                                    0x5F3759DF, AO.mult, AO.add)
            # one 2nd-order Householder step: inv = y*(15/8 - 5/4 w + 3/8 w^2),
            # w = var*y^2; error O(e0^3) ~ 1e-4 rel
            w = t1
            nc.vector.tensor_scalar(w[:], yv[:], yv[:, 0:1], var[:, 0:1],
                                    AO.mult, AO.mult)
            u2 = var  # reuse
            nc.vector.tensor_scalar(u2[:], w[:], 0.375, -1.25, AO.mult, AO.add)
            nc.vector.tensor_scalar(u2[:], u2[:], w[:, 0:1], 1.875, AO.mult, AO.add)
            nc.vector.tensor_scalar(inv[:], yv[:], u2[:, 0:1], None, AO.mult)

            # ---- normalize + sigmoid + output, per half ----
            sig = wk.tile([BL, H], F32)
            for nt in range(2):
                cs = slice(nt * 512, (nt + 1) * 512)
                nc.vector.tensor_scalar(pre16[:, cs], pre16[:, cs], mu[:, 0:1],
                                        inv[:, 0:1], AO.subtract, AO.mult)
                if not ln_trivial:
                    nc.vector.tensor_tensor(pre16[:, cs], pre16[:, cs],
                                            lngb[:, cs], AO.mult)
                    nc.vector.tensor_tensor(pre16[:, cs], pre16[:, cs],
                                            lngb[:, H + nt * 512:H + (nt + 1) * 512],
                                            AO.add)
                nc.scalar.activation(sig[:, cs], pre16[:, cs],
                                     mybir.ActivationFunctionType.Sigmoid)
                for q in range(2):
                    qs = slice(nt * 512 + q * 256, nt * 512 + (q + 1) * 256)
                    eng = nc.sync if q == 0 else nc.scalar
                    eng.dma_start(y[:, qs], sig[:, qs])
    if split:
        split_multi_waits(nc)
    return nc


def _prep_host(x, h_prev, mem_tensor, W_w, W_b, U_w, U_b, M_w, M_b,
               Qr_w, Qr_b, Ql_w, Ql_b, ln_g, ln_b):
    shared = {}
    mwt = _chunked_T(np.ascontiguousarray(M_w.T))
    for m, W in enumerate((W_w, U_w, Qr_w, Ql_w)):
        ct = _chunked_T(np.ascontiguousarray(W.T))
        shared[f"wh{m}"] = _n0_first(ct.astype(ml_dtypes.bfloat16))
    bst = np.stack([W_b, U_b, Qr_b, Ql_b])
    bh, bl = _split_bf16(bst)
    bpair = np.ones((8, H + 16), ml_dtypes.bfloat16)
    bpair[0:4, 0:H] = bh
    bpair[4:8, 0:H] = bl
    shared["bpair"] = np.ascontiguousarray(bpair)

    powmat = np.zeros((2 * NB, 2), np.float32)
    powmat[:NB, 0] = 2.0 ** np.arange(NB - 1, -1, -1)
    powmat[NB:, 1] = 2.0 ** np.arange(NB - 1, -1, -1)
    cfc = np.zeros((128, 344), np.float32)
    cfc[:, 128:288] = mwt
    cfc[0:16, 288:304] = np.eye(16)
    cfc[0:20, 304:306] = powmat
    cfc[0:16, 306:307] = np.arange(BL)[:, None]
    cfc[0:1, 307:327] = M_b[None, :]
    cfc[0:8, 327:343] = 1.0
    shared["cfc_base"] = cfc

    shared["ln_trivial"] = bool(np.all(ln_g == 1.0) and np.all(ln_b == 0.0))
    if not shared["ln_trivial"]:
        lngb = np.zeros((BL, 2 * H), np.float32)
        lngb[:, 0:H] = ln_g[None, :]
        lngb[:, H:2 * H] = ln_b[None, :]
        shared["lngb"] = lngb

    per_core = []
    for c in range(NC):
        bs = slice(c * BL, (c + 1) * BL)
        d = {k: v for k, v in shared.items() if not k.startswith("_") and
             k not in ("cfc_base", "ln_trivial")}
        xt = _chunked_T(np.ascontiguousarray(x[bs].T))
        ht = _chunked_T(np.ascontiguousarray(h_prev[bs].T))
        cbc = np.zeros((128, 256), ml_dtypes.bfloat16)
        cbc[:, 0:128] = xt.astype(ml_dtypes.bfloat16)
        cbc[:, 128:256] = ht.astype(ml_dtypes.bfloat16)
        d["cbc"] = np.ascontiguousarray(cbc)
        cfc_c = shared["cfc_base"].copy()
        cfc_c[:, 0:128] = ht
        d["cfc"] = cfc_c
        d["mem"] = np.ascontiguousarray(mem_tensor[:, bs, :]).reshape(T * BL, H)
        per_core.append(d)
    return per_core, shared["ln_trivial"]


def kernel(**inputs):
    in_maps, ln_trivial = _prep_host(**{k: np.asarray(v) for k, v in inputs.items()})
    key = ("nc", ln_trivial)
    if key not in _CACHED:
        _CACHED[key] = build(ln_trivial=ln_trivial)
    nc = _CACHED[key]
    res = run_bass_kernel_spmd(nc, in_maps, list(range(NC)),
                               trace=bool(_CACHED.get("trace")))
    _CACHED["last_result"] = res
    return np.concatenate([res.results[c]["y"] for c in range(NC)], axis=0)
